# revision 75
# baseline (speedup 1.0000x reference)
"""GQA attention (RoPE, causal) for nn_Attention_43293270343986 on 8 TRN2 cores.

Sharding: tensor-parallel over the 8 KV head groups. Core c owns kv head c and
its 4 query heads (Wq/Wk/Wv column-sharded, Wo row-sharded); the host sums the
8 partial outputs (the all-reduce equivalent).

v2 design notes (cost-model driven; the graded time == TimelineSim):
- Projections run as fp8e4m3 DoubleRow matmuls with residual compensation:
    X@W ~= (X8 + RX8)@W8 + X8@RW8
  where X8/RX8 (hidden + its quantization residual) come from the host and
  W8/RW8 are host-prepared (weights pre-scaled by 256 so e4m3 sees normal
  range; the descale is folded into the RoPE cos/sin tables). DoubleRow packs
  K=256 per instruction at 0.5 cyc/row -> 3 terms cost 0.75x of f16.
- Scores: f16 (exact) or optional fp8 DoubleRow over a dh-split [32,2,...]
  layout (QK_MODE="dr1", ~1.7e-2 rel err; off by default - it only buys ~2us
  net and burns the whole error budget).
- The causal mask is ADDED to diagonal score blocks on PE (maskt.T @ I
  accumulated into the score PSUM with -60000 fills) so exp -> PV needs no
  intervening vector op.
- P@V is restructured with P^T as the stationary operand and [V|1] moving
  (65 rows instead of 512 per block): output lands token-major [q, dh+1]
  with the softmax denominator in column 64 -> per-partition reciprocal +
  scalar-multiply normalize, no gpsimd broadcast / DMA hops. PE transposes
  (with an on-device identity) restore the feature-major O^T for Wo.
- Wo stays f16 (fp8 there fails the 2e-2 gate; residual-compensating the
  activations costs more engine time than it saves). Each window's Wo is
  emitted one window LATE so PE never waits on the normalize chain, and the
  final window borrows the idle score-PSUM slots + the Act engine.
- PSUM pending-zero gotcha: start=True arms the WHOLE bank, so the pso
  accumulator banks are pre-zeroed with 0x0 DoubleRow matmuls and all PV
  matmuls accumulate with start=False.
- Output yt is written f16 (halves output DMA); host sums partials in fp32.
"""

from contextlib import ExitStack

import numpy as np

import concourse.bacc as bacc
import concourse.mybir as mybir
import concourse.tile as tile
from concourse.bass_utils import run_bass_kernel_spmd
from concourse.masks import make_identity

B, S, D = 2, 2048, 2048
HQ, HKV, DH = 32, 8, 64
ROPE_THETA = 10000.0

N_CORES = 8
T = B * S               # 4096 tokens (batch-major concat)
NQH = HQ // HKV         # 4 query heads per core
QC = NQH * DH           # 256 q-projection cols per core
TT = 512                # token tile (matmul moving dim)
KC = D // 128           # 16 contraction chunks for the projections
KH = KC // 2            # ht is loaded in two halves of 8 chunks
NTAU = T // TT          # 8 token tiles
QW = S // TT            # 4 query windows per batch
NKB = S // 128          # 16 key blocks per batch

PROJ_MODE = "dr3"       # "dr3" (fp8 DoubleRow, residual-compensated) | "f16"
QK_MODE = "f16"         # "f16" | "dr1" (fp8 DoubleRow scores)
DEBUG_DUMP = False      # add qt/ktv/v/ot debug outputs

WSCALE = 256.0          # weight prescale for e4m3 (descaled via rope tables)
QKSCALE = 16.0          # q/k prescale entering the score matmul

_F32 = mybir.dt.float32
_F16 = mybir.dt.float16
_F8 = mybir.dt.float8e4
_DR = mybir.MatmulPerfMode.DoubleRow


def build_nc():
    sdt = _F16
    proj_dr = PROJ_MODE == "dr3"
    qk_dr = QK_MODE == "dr1"

    nc = bacc.Bacc("TRN2", target_bir_lowering=False, debug=False,
                   num_devices=N_CORES)

    if proj_dr:
        h8_d = nc.dram_tensor("h8", [D, T], _F8, kind="ExternalInput")
        rh8_d = nc.dram_tensor("rh8", [D, T], _F8, kind="ExternalInput")
        wq_d = nc.dram_tensor("wq", [2, D, QC], _F8, kind="ExternalInput")
        wkv_d = nc.dram_tensor("wkv", [2, D, 128], _F8, kind="ExternalInput")
    else:
        h8_d = nc.dram_tensor("h8", [D, T], sdt, kind="ExternalInput")
        wq_d = nc.dram_tensor("wq", [1, D, QC], sdt, kind="ExternalInput")
        wkv_d = nc.dram_tensor("wkv", [1, D, 128], sdt, kind="ExternalInput")
    wo_d = nc.dram_tensor("wo", [QC, D], sdt, kind="ExternalInput")
    cos2_d = nc.dram_tensor("cos2", [128, S], sdt, kind="ExternalInput")
    sin2_d = nc.dram_tensor("sin2", [128, S], sdt, kind="ExternalInput")
    coskv_d = nc.dram_tensor("coskv", [128, S], sdt, kind="ExternalInput")
    sinkv_d = nc.dram_tensor("sinkv", [128, S], sdt, kind="ExternalInput")
    idhi_d = nc.dram_tensor("idhi", [128, DH], sdt, kind="ExternalInput")
    maskt_d = nc.dram_tensor("maskt", [128, 128], sdt, kind="ExternalInput")
    yt_d = nc.dram_tensor("yt", [D, T], sdt, kind="ExternalOutput")
    if DEBUG_DUMP:
        qt_dbg = nc.dram_tensor("qt_dbg", [128, 2, T], sdt,
                                kind="ExternalOutput")
        ktv_dbg = nc.dram_tensor("ktv_dbg", [128, T], sdt,
                                 kind="ExternalOutput")
        v_dbg = nc.dram_tensor("v_dbg", [128, T // 128, DH + 1], sdt,
                               kind="ExternalOutput")
        ot_dbg = nc.dram_tensor("ot_dbg", [128, 2, T], sdt,
                                kind="ExternalOutput")

    nterm = 2 if proj_dr else 1
    h_r = h8_d[:].rearrange("(ko p) t -> p ko t", p=128)
    rh_r = rh8_d[:].rearrange("(ko p) t -> p ko t", p=128) if proj_dr else None
    yt_r = yt_d[:].rearrange("(m p) t -> p m t", p=128)

    with tile.TileContext(nc) as tc:
        es_o = ExitStack()
        es_qkv = ExitStack()
        op = es_o.enter_context(tc.tile_pool(name="op", bufs=1))
        qkvp = es_qkv.enter_context(tc.tile_pool(name="qkvp", bufs=1))
        # maskt[c, k] = M[k, c] with M[k, q] = 0 if k <= q else -60000; the
        # causal mask is ADDED to diagonal score blocks via M.T @ I on PE
        maskt_sb = op.tile([128, 128], sdt, name="maskt_sb")
        id128_sb = op.tile([128, 128], sdt, name="id128_sb")
        wo_sb = op.tile([128, 2, D], sdt, name="wo_sb")
        zrow = op.tile([1, 2, TT], _F8, name="zrow")
        if qk_dr:
            q8_all = qkvp.tile([32, 2, NQH, T], _F8)
            k8_all = qkvp.tile([32, 2, T], _F8)
        else:
            qT_all = qkvp.tile([128, 2, T], sdt)   # head pairs on partitions
            kt_hi = qkvp.tile([128, T], sdt)       # kT copy on rows 64-127
        ktv = qkvp.tile([128, T], sdt)             # kT rows 0-63, vT 64-127
        v_all = qkvp.tile([128, T // 128, DH + 1], sdt)

        # ---------------- Phase A: QKV projections + RoPE ----------------
        with (
            tc.tile_pool(name="aconsts", bufs=1) as aconsts,
            tc.tile_pool(name="htp", bufs=3) as htp,
            tc.tile_pool(name="atmp", bufs=3) as atmp,
            tc.tile_pool(name="apsum", bufs=6, space="PSUM") as apsum,
            tc.tile_pool(name="tpsum", bufs=2, space="PSUM") as tpsum,
        ):
            wq_sb = aconsts.tile([128, nterm, KC, QC], _F8 if proj_dr else sdt)
            wkv_sb = aconsts.tile([128, nterm, KC, 128], _F8 if proj_dr else sdt)
            cos2_sb = aconsts.tile([128, S], sdt)
            sin2_sb = aconsts.tile([128, S], sdt)
            coskv_sb = aconsts.tile([128, S], sdt)
            sinkv_sb = aconsts.tile([128, S], sdt)
            idhi_sb = aconsts.tile([128, DH], sdt)
            wq_r = wq_d[:].rearrange("s (ko p) c -> p s ko c", p=128)
            for s in range(nterm):
                nc.sync.dma_start(wq_sb[:, s, 0:KH, :], wq_r[:, s, 0:KH, :])
                with tc.high_priority(offset=-60):
                    nc.sync.dma_start(wq_sb[:, s, KH:KC, :],
                                      wq_r[:, s, KH:KC, :])
            nc.any.memset(v_all[:, :, DH:DH + 1], 1.0)
            nc.any.memset(zrow, 0.0)
            make_identity(nc, id128_sb)

            for tau in range(NTAU):
                tok = tau * TT
                pos = (tau % QW) * TT

                hts = []
                for half in range(2):
                    ht = htp.tile([128, KH, TT], _F8 if proj_dr else sdt,
                                  tag=f"ht{half}")
                    nc.sync.dma_start(
                        ht, h_r[:, half * KH:(half + 1) * KH, tok:tok + TT])
                    hts.append(ht)
                rhts = []
                if proj_dr:
                    for half in range(2):
                        rht = htp.tile([128, KH, TT], _F8, tag=f"rht{half}")
                        nc.gpsimd.dma_start(
                            rht,
                            rh_r[:, half * KH:(half + 1) * KH, tok:tok + TT])
                        rhts.append(rht)
                if tau == 1:
                    nc.sync.dma_start(maskt_sb, maskt_d[:])
                if tau == 6:
                    # prefetch Wo while the DMA track drains phase-A loads
                    nc.sync.dma_start(
                        wo_sb, wo_d[:].rearrange("(c p) n -> p c n", p=128))
                if tau == 0:
                    wkv_r = wkv_d[:].rearrange("s (ko p) c -> p s ko c", p=128)
                    for s in range(nterm):
                        nc.sync.dma_start(wkv_sb[:, s], wkv_r[:, s])
                    with tc.high_priority(offset=-150):
                        nc.sync.dma_start(cos2_sb, cos2_d[:])
                    with tc.high_priority(offset=-250):
                        nc.sync.dma_start(sin2_sb, sin2_d[:])
                    with tc.high_priority(offset=-300):
                        nc.sync.dma_start(coskv_sb, coskv_d[:])
                        nc.sync.dma_start(sinkv_sb, sinkv_d[:])
                    nc.sync.dma_start(idhi_sb, idhi_d[:])

                def proj(w_sb, wcols, np_):
                    ps = apsum.tile([np_, TT], _F32, tag="pa")
                    if proj_dr:
                        # (X8+RX8)@W8 + X8@RW8, DoubleRow over chunk pairs
                        steps = []
                        for jj in range(KH):
                            half, d = jj // (KH // 2), (2 * jj) % KH
                            steps.append((w_sb[:, 0, 2 * jj:2 * jj + 2, wcols],
                                          hts[half][:, d:d + 2, :]))
                            steps.append((w_sb[:, 0, 2 * jj:2 * jj + 2, wcols],
                                          rhts[half][:, d:d + 2, :]))
                            steps.append((w_sb[:, 1, 2 * jj:2 * jj + 2, wcols],
                                          hts[half][:, d:d + 2, :]))
                        for i, (w, x) in enumerate(steps):
                            nc.tensor.matmul(ps, w, x, start=(i == 0),
                                             stop=(i == len(steps) - 1),
                                             perf_mode=_DR)
                    else:
                        for k in range(KC):
                            nc.tensor.matmul(
                                ps, w_sb[:, 0, k, wcols],
                                hts[k // KH][:, k % KH, :],
                                start=(k == 0), stop=(k == KC - 1))
                    return ps

                # head dims are host-permuted so rotate-half pairs sit on
                # adjacent partitions: the rotation is a neighbor swap
                # (intra-quadrant stream_shuffle on DVE), sign folded into sin
                swap_mask = [i ^ 1 for i in range(32)]

                def rope_pair(w_sb, wcols, dst, cs, sn):
                    ps = proj(w_sb, wcols, 128)
                    raw = atmp.tile([128, TT], sdt, tag="raw")
                    nc.scalar.copy(raw, ps)
                    rot = atmp.tile([128, TT], sdt, tag="rot")
                    nc.vector.stream_shuffle(rot, raw, swap_mask)
                    tcos = atmp.tile([128, TT], sdt, tag="tcos")
                    nc.vector.tensor_mul(tcos, raw, cs)
                    tsin = atmp.tile([128, TT], sdt, tag="tsin")
                    nc.vector.tensor_mul(tsin, rot, sn)
                    nc.vector.tensor_add(dst, tcos, tsin)

                for j in range(2):
                    if qk_dr:
                        qf8 = atmp.tile([128, TT], _F8, tag="qf8")
                        rope_pair(wq_sb, slice(j * 128, (j + 1) * 128), qf8,
                                  cos2_sb[:, pos:pos + TT],
                                  sin2_sb[:, pos:pos + TT])
                        for jj in range(2):
                            for dhh in range(2):
                                eng = (nc.scalar, nc.sync, nc.scalar,
                                       nc.gpsimd)[2 * jj + dhh]
                                eng.dma_start(
                                    q8_all[:, dhh, 2 * j + jj, tok:tok + TT],
                                    qf8[64 * jj + 32 * dhh:
                                        64 * jj + 32 * dhh + 32, :])
                    else:
                        rope_pair(wq_sb, slice(j * 128, (j + 1) * 128),
                                  qT_all[:, j, tok:tok + TT],
                                  cos2_sb[:, pos:pos + TT],
                                  sin2_sb[:, pos:pos + TT])
                # packed K|V: rows 0-63 get RoPE'd K (cos/sin), rows 64-127
                # get V scaled by 1/WSCALE (coskv=1/WSCALE, sinkv=0 there)
                rope_pair(wkv_sb, slice(0, 128), ktv[:, tok:tok + TT],
                          coskv_sb[:, pos:pos + TT], sinkv_sb[:, pos:pos + TT])
                if qk_dr:
                    k8t = atmp.tile([64, TT], _F8, tag="k8t")
                    nc.vector.tensor_copy(k8t, ktv[0:64, tok:tok + TT])
                    for dhh in range(2):
                        eng = (nc.scalar, nc.sync)[dhh]
                        eng.dma_start(k8_all[:, dhh, tok:tok + TT],
                                      k8t[32 * dhh:32 * dhh + 32, :])
                else:
                    # duplicate kT onto rows 64-127 for the odd heads
                    nc.gpsimd.dma_start(kt_hi[64:128, tok:tok + TT],
                                        ktv[0:64, tok:tok + TT])
                # V: PE-transpose rows 64-127 of ktv to token-major
                for c in range(TT // 128):
                    pvt = tpsum.tile([128, DH], sdt, tag="pvt")
                    nc.tensor.transpose(
                        pvt, ktv[64:128, tok + c * 128:tok + (c + 1) * 128],
                        idhi_sb[64:128, :])
                    nc.vector.tensor_copy(
                        v_all[:, tau * (TT // 128) + c, 0:DH], pvt)

        # ------- Phase B: attention + fused output projection -------
        # Scores land as S^T [keys, 2 heads, queries] per 128-key block; one
        # exp covers the head pair. P^T blocks then act as the STATIONARY
        # operand of the P@V matmul with [V|1] moving (65 rows), so O arrives
        # token-major with the softmax denominator in column 64. A per-
        # partition reciprocal+scale normalizes, PE transposes restore O^T,
        # and the window's Wo matmuls run immediately after.
        oT_all = op.tile([128, 2, T], sdt)
        sc_exp = float(1.0 / (np.sqrt(DH) * QKSCALE * QKSCALE))
        with (
            tc.tile_pool(name="bconsts", bufs=1) as bconsts,
            tc.tile_pool(name="ptp", bufs=6) as ptp,
            tc.tile_pool(name="btmp", bufs=3) as btmp,
            tc.tile_pool(name="yp", bufs=6) as yp,
            tc.tile_pool(name="spsum", bufs=2, space="PSUM") as spsum,
            tc.tile_pool(name="bpsum", bufs=1, space="PSUM") as bpsum,
            tc.tile_pool(name="ypsum", bufs=2, space="PSUM") as ypsum,
        ):
            # copy + DMA-issue engine per 2-m-block group: spread so no
            # single queue serializes the window
            cengs = ["v", "v", "v", "v", "v", "v", "v", "v"]
            dengs = ["s", "p", "s", "p", "s", "p", "s", "p"]

            def emit_wo(tok0, final=False, act_help=False):
                # output projection for one token window, two m-blocks per
                # PSUM tile / copy / DMA. The final window additionally
                # borrows the idle score slots and the Act engine so its
                # serial tail drains ~2x faster.
                for m in range(D // 128):
                    if final and m % 2 == 1:
                        psy = spsum.tile([128, TT], _F32, tag="pss",
                                         name="psy")
                    else:
                        psy = ypsum.tile([128, TT], _F32, tag="psy",
                                         name="psy")
                    for ch in range(2):
                        nc.tensor.matmul(
                            psy, wo_sb[:, ch, m * 128:(m + 1) * 128],
                            oT_all[:, ch, tok0:tok0 + TT],
                            start=(ch == 0), stop=(ch == 1))
                    ysb = yp.tile([128, TT], sdt, tag="ysb")
                    ce = ("a" if ((final or act_help) and m % 2 == 1)
                          else cengs[m % 8])
                    if ce == "a":
                        nc.scalar.copy(ysb, psy)
                    else:
                        ceng = nc.vector if ce == "v" else nc.gpsimd
                        ceng.tensor_copy(ysb, psy)
                    deng = {"s": nc.sync, "p": nc.gpsimd,
                            "a": nc.scalar}[dengs[m % 8]]
                    deng.dma_start(
                        yt_r[:, m:m + 1, tok0:tok0 + TT], ysb)

            prev_tok0 = None
            for b in range(B):
                for qw in range(QW):
                    tok0 = b * S + qw * TT
                    nkb = (TT // 128) * (qw + 1)
                    # layout [q, ch, qb, j*64+dh]: the transpose reads a
                    # contiguous [128, 128] chunk per (ch, qb)
                    o_sb = btmp.tile([128, 2, QW, 128], sdt, tag="osb")
                    for hp in range(2):          # head pair = (2*hp, 2*hp+1)
                        pso = bpsum.tile([128, 2 * QW, 128], _F32, tag="pso")
                        # PSUM start=True arms a pending-zero for the WHOLE
                        # bank, so interleaved accumulation groups in one
                        # bank corrupt each other. Pre-zero both pso banks
                        # with 0x0 DoubleRow matmuls (cheap); all PV matmuls
                        # then accumulate with start=False.
                        for half in range(2):
                            nc.tensor.matmul(
                                pso[:, 4 * half:4 * half + 4, :].rearrange(
                                    "p a b -> p (a b)"),
                                zrow[:, :, 0:128], zrow[:, :, 0:TT],
                                start=True, stop=True, perf_mode=_DR)

                        def pv_step(kb, pt):
                            r = kb - (TT // 128) * qw
                            for i in range(2):
                                for qb in range(max(r, 0), QW):
                                    nc.tensor.matmul(
                                        pso[:, i * QW + qb, 0:DH + 1],
                                        pt[:, i, qb * 128:(qb + 1) * 128],
                                        v_all[:, b * NKB + kb, :],
                                        start=False,
                                        stop=(kb == (TT // 128) * qw + qb),
                                        skip_group_check=True)

                        # software-pipelined: scores(kb+1) issue before
                        # PV(kb) so PE fills the exp(kb) latency
                        pt_prev = None
                        for kb in range(nkb):
                            kt0 = b * S + kb * 128
                            r = kb - (TT // 128) * qw
                            w = 128 * r if r >= 0 else 0  # fully-masked cols
                            pss = spsum.tile([128, 2, TT], _F32, tag="pss")
                            pt = ptp.tile([128, 2, TT], sdt, tag="pt")
                            for i in range(2):
                                if qk_dr:
                                    nc.tensor.matmul(
                                        pss[:, i, w:],
                                        k8_all[:, :, kt0:kt0 + 128],
                                        q8_all[:, :, 2 * hp + i,
                                               tok0 + w:tok0 + TT],
                                        start=True, stop=True, perf_mode=_DR)
                                else:
                                    off = i * 64
                                    ksrc = ktv if i == 0 else kt_hi
                                    nc.tensor.matmul(
                                        pss[:, i, w:],
                                        ksrc[off:off + 64, kt0:kt0 + 128],
                                        qT_all[off:off + 64, hp,
                                               tok0 + w:tok0 + TT],
                                        start=True, stop=(r < 0))
                                if r >= 0:
                                    # fold the causal mask into the diagonal
                                    # block on PE: accumulate M = maskt.T @ I
                                    nc.tensor.matmul(
                                        pss[:, i, w:w + 128],
                                        maskt_sb, id128_sb,
                                        start=False, stop=True,
                                        skip_group_check=True)
                            nc.scalar.activation(
                                pt[:, :, w:], pss[:, :, w:],
                                mybir.ActivationFunctionType.Exp,
                                scale=sc_exp)
                            if pt_prev is not None:
                                pv_step(kb - 1, pt_prev)
                            pt_prev = pt
                        pv_step(nkb - 1, pt_prev)
                        rec = btmp.tile([128, 2 * QW], _F32, tag="rec")
                        nc.vector.reciprocal(rec, pso[:, :, DH:DH + 1])
                        for i in range(2):
                            for qb in range(QW):
                                idx = i * QW + qb
                                nc.vector.tensor_scalar_mul(
                                    o_sb[:, hp, qb, i * DH:(i + 1) * DH],
                                    pso[:, idx, 0:DH],
                                    rec[:, idx:idx + 1])
                    # the PREVIOUS window's Wo runs here: its oT is long
                    # ready, so PE never waits on this window's normalize.
                    # In short windows Act has exp slack -> let it copy.
                    if prev_tok0 is not None:
                        emit_wo(prev_tok0)
                    # transpose O back to feature-major [2*128, TT] chunks
                    # (tt shares the psy pool slot; both are window-scoped)
                    for ch in range(2):
                        tt = ypsum.tile([128, QW, 128], sdt, tag="psy",
                                        name="tt")
                        for qb in range(QW):
                            nc.tensor.transpose(
                                tt[:, qb, :], o_sb[:, ch, qb, :], id128_sb)
                        nc.vector.tensor_copy(
                            oT_all[:, ch, tok0:tok0 + TT], tt)
                    prev_tok0 = tok0
            emit_wo(prev_tok0, final=True)
            if DEBUG_DUMP:
                nc.sync.dma_start(qt_dbg[:], qT_all)
                nc.sync.dma_start(ktv_dbg[:], ktv)
                nc.sync.dma_start(v_dbg[:], v_all)
                nc.sync.dma_start(ot_dbg[:], oT_all)

        es_qkv.close()
        es_o.close()

    nc.compile()
    return nc


def _rope_tables():
    inv_freq = 1.0 / (ROPE_THETA ** (np.arange(0, DH, 2, dtype=np.float64) / DH))
    t = np.arange(S, dtype=np.float64)
    freqs = np.outer(t, inv_freq)
    emb = np.concatenate([freqs, freqs], axis=-1)      # [S, DH]
    return (np.cos(emb).astype(np.float32), np.sin(emb).astype(np.float32))


def _np_f8():
    import ml_dtypes
    return np.dtype(ml_dtypes.float8_e4m3)


def _q8(x):
    return np.asarray(x, np.float32).astype(_np_f8())


def make_in_maps(hidden_states, Wq, Wk, Wv, Wo):
    proj_dr = PROJ_MODE == "dr3"
    ndt = np.dtype(np.float16)
    f8 = _np_f8()

    hT = np.ascontiguousarray(
        np.asarray(hidden_states, np.float32).reshape(T, D).T)
    cos, sin = _rope_tables()                          # [S, DH]
    # permutation: new position 2i holds dim i, 2i+1 holds dim i+32, so the
    # rotate-half partner is always the adjacent partition
    operm = np.empty(DH, np.int64)
    operm[0::2] = np.arange(DH // 2)
    operm[1::2] = np.arange(DH // 2) + DH // 2
    sgn = np.where(np.arange(DH) % 2 == 0, -1.0, 1.0)[:, None]  # [DH,1]
    # q/k rows: descale the WSCALE'd projection, then prescale by QKSCALE
    rs = QKSCALE / WSCALE if proj_dr else QKSCALE
    cosp = cos.T[operm] * rs                           # [DH, S] permuted
    sinp = sin.T[operm] * sgn * rs                     # sign-folded
    vdescale = (1.0 / WSCALE) if proj_dr else 1.0
    cos2 = np.ascontiguousarray(
        np.concatenate([cosp, cosp], axis=0)).astype(ndt)  # [128, S]
    sin2 = np.ascontiguousarray(
        np.concatenate([sinp, sinp], axis=0)).astype(ndt)
    coskv = np.ascontiguousarray(np.concatenate(
        [cosp, np.full((64, S), vdescale)], axis=0)).astype(ndt)
    sinkv = np.ascontiguousarray(
        np.concatenate([sinp, np.zeros((64, S))], axis=0)).astype(ndt)
    idhi = np.zeros((128, DH), np.float32)
    idhi[64:128, :] = np.eye(DH, dtype=np.float32)
    idhi = idhi.astype(ndt)

    kk = np.arange(128)[:, None]
    qq = np.arange(128)[None, :]
    mm = np.where(kk <= qq, 0.0, -60000.0).astype(np.float32)  # M[k, q]
    maskt = np.ascontiguousarray(mm.T).astype(ndt)             # [c=q, k]

    Wq = np.asarray(Wq, np.float32)
    Wk = np.asarray(Wk, np.float32)
    Wv = np.asarray(Wv, np.float32)
    Wo = np.asarray(Wo, np.float32)

    if proj_dr:
        h8 = _q8(hT)
        rh8 = _q8(hT - h8.astype(np.float32))

    def wsplit(w):
        # [D, C] -> [2, D, C] fp8: (W8, RW8) with WSCALE prescale
        wt = w * WSCALE
        w8 = _q8(wt)
        rw8 = _q8(wt - w8.astype(np.float32))
        return np.ascontiguousarray(np.stack([w8, rw8], axis=0))

    in_maps = []
    for c in range(N_CORES):
        wq_c = np.ascontiguousarray(
            Wq[:, c * QC:(c + 1) * QC].reshape(D, NQH, DH)[:, :, operm]
            .reshape(D, QC))
        wkv_c = np.ascontiguousarray(np.concatenate(
            [Wk[:, c * DH:(c + 1) * DH][:, operm],
             Wv[:, c * DH:(c + 1) * DH]],
            axis=1))
        wo_c = np.ascontiguousarray(Wo[c * QC:(c + 1) * QC, :]).astype(ndt)
        m = {
            "wo": wo_c, "cos2": cos2, "sin2": sin2, "coskv": coskv,
            "sinkv": sinkv, "idhi": idhi, "maskt": maskt,
        }
        if proj_dr:
            m["h8"] = h8
            m["rh8"] = rh8
            m["wq"] = wsplit(wq_c)
            m["wkv"] = wsplit(wkv_c)
        else:
            m["h8"] = hT.astype(ndt)
            m["wq"] = np.ascontiguousarray(wq_c[None]).astype(ndt)
            m["wkv"] = np.ascontiguousarray(wkv_c[None]).astype(ndt)
        in_maps.append(m)
    return in_maps


def postprocess(results):
    acc = np.zeros((D, T), np.float32)
    for res in results:
        acc += np.asarray(res["yt"], dtype=np.float32)
    return np.ascontiguousarray(acc.T).reshape(B, S, D)


def kernel(hidden_states, Wq, Wk, Wv, Wo):
    nc = build_nc()
    in_maps = make_in_maps(hidden_states, Wq, Wk, Wv, Wo)
    res = run_bass_kernel_spmd(nc, in_maps, core_ids=list(range(N_CORES)))
    return postprocess(res.results)
